# revision 1
# baseline (speedup 1.0000x reference)
"""CapsuleLinear (k-means routing) Trainium2 kernel.

Math: priors[b,o,i,j] = sum_l w[o,j,l] x[b,i,l];  3 rounds of k-means routing
over in_capsules, squash=False.

Key reformulation: priors never needs to be materialized.  With
G_o = W_o^T W_o (64x64 per out-capsule, precomputed on-device once):

    u_0[b,l]   = sum_i x[b,i,l]                  (scale of u is irrelevant)
    per iter:  p = G_o u;  v = p / sqrt(u.p)     (v = W^T out_normalized)
               logits[i,o] = sum_l x[b,i,l] v[o,l]
               e = exp(logits)                   (softmax Z cancels in v)
               u[o,l] = sum_i e[i,o] x[b,i,l];  Z[o] = sum_i e[i,o]
    output:    out[b,o,:] = W_o u_3[o,:] / Z_3[o]

This turns a 536MB priors tensor + 6 passes over it into ~35us of
TensorEngine work per core.  Sharding: data-parallel over batch,
4 samples/core x 8 cores, weight replicated, no collectives.
"""

import sys

if "/opt/trn_rl_repo" not in sys.path:
    sys.path.insert(0, "/opt/trn_rl_repo")

from contextlib import ExitStack

import numpy as np

import concourse.bacc as bacc
import concourse.bass as bass
import concourse.bass_utils as bass_utils
import concourse.mybir as mybir
import concourse.tile as tile
from concourse.masks import make_identity

BF = mybir.dt.bfloat16
F32 = mybir.dt.float32
AF = mybir.ActivationFunctionType
ALU = mybir.AluOpType

B_GLOBAL = 32
N_CORES = 8
B_LOC = B_GLOBAL // N_CORES  # 4 samples per core
O = 128   # out_capsules
I = 512   # in_capsules
J = 64    # out_length
L = 64    # in_length
C = 4     # i-chunks of 128
NITER = 3


def _body(ctx: ExitStack, tc: "tile.TileContext", x_d, w_d, out_d, probe=None):
    nc = tc.nc

    const_pool = ctx.enter_context(tc.tile_pool(name="const", bufs=1))
    big_pool = ctx.enter_context(tc.tile_pool(name="big", bufs=1))
    sb = ctx.enter_context(tc.tile_pool(name="sb", bufs=2))
    sbE = ctx.enter_context(tc.tile_pool(name="sbE", bufs=3))
    psA = ctx.enter_context(tc.tile_pool(name="psA", bufs=2, space="PSUM"))
    psL = ctx.enter_context(tc.tile_pool(name="psL", bufs=2, space="PSUM"))
    psU = ctx.enter_context(tc.tile_pool(name="psU", bufs=2, space="PSUM"))
    psT = ctx.enter_context(tc.tile_pool(name="psT", bufs=2, space="PSUM"))

    # ---- constants ----
    ident_bf = const_pool.tile([128, 128], BF)
    make_identity(nc, ident_bf[:])
    ident_f = const_pool.tile([128, 128], F32)
    make_identity(nc, ident_f[:])
    ones_col = const_pool.tile([128, 1], BF)
    nc.vector.memset(ones_col[:], 1.0)
    ones_row = const_pool.tile([1, 128], BF)
    nc.vector.memset(ones_row[:], 1.0)

    # ---- load x first (small, 1KB descriptors: partition p holds rows
    # i=4p..4p+3; the i->partition remap is irrelevant: every i-reduction
    # is order-free and x_bf/xT use the same mapping) ----
    x_f32 = big_pool.tile([128, B_LOC, C, L], F32)
    x_bf = big_pool.tile([128, B_LOC, C, L + 1], BF)
    xT_sb = big_pool.tile([L, B_LOC, C, 128], BF)
    x_view = x_d.rearrange("b (p r) l -> p b r l", p=128, r=C)
    nc.sync.dma_start(x_f32[:], x_view)

    # ---- load weight as (j, o, l) in 8 chunks, one TILE per chunk so the
    # G matmuls of chunk k depend only on DMA k (per-tile dep tracking) ----
    w_jol = w_d.transpose([1, 0, 2])  # (j, o, l) view of DRAM (o, j, l)
    OBLK = 16
    w_tiles = []
    wb_tiles = []
    for k in range(O // OBLK):
        w_k = big_pool.tile([J, OBLK, L], F32, tag=f"w_{k}")
        nc.sync.dma_start(w_k[:], w_jol[:, bass.ts(k, OBLK), :])
        w_tiles.append(w_k)
        wb_k = big_pool.tile([J, OBLK, L], BF, tag=f"wb_{k}")
        nc.vector.tensor_copy(wb_k[:], w_k[:])
        wb_tiles.append(wb_k)

    def w_ap(o):
        return w_tiles[o // OBLK][:, o % OBLK, :]

    def wb_ap(o):
        return wb_tiles[o // OBLK][:, o % OBLK, :]
    nc.vector.tensor_copy(x_bf[:, :, :, :L], x_f32[:])
    nc.vector.memset(x_bf[:, :, :, L], 1.0)
    for b in range(B_LOC):
        for c in range(C):
            xt_ps = psT.tile([L, 128], BF, tag="tr")
            nc.tensor.transpose(xt_ps[:], x_bf[:, b, c, :L], ident_bf[:])
            if (b * C + c) % 2 == 0:
                nc.scalar.copy(xT_sb[:, b, c, :], xt_ps[:])
            else:
                nc.vector.tensor_copy(xT_sb[:, b, c, :], xt_ps[:])

    def _dummy_out():
        nc.sync.dma_start(out_d[0], x_f32[:, 0, 0, :])

    if probe == "P0":
        _dummy_out()
        return

    if probe == "P1":
        _dummy_out()
        return

    # ---- G_o = W_o^T W_o per out-capsule (WT deferred to the final phase) ----
    WT_sb = big_pool.tile([L, O, J], BF)  # (l, o, j)
    GB = 8
    G_tiles = []
    for g in range(O // GB):
        g_ps = psA.tile([L, GB, L], F32, tag="pT")
        for oo in range(GB):
            o = g * GB + oo
            nc.tensor.matmul(g_ps[:, oo, :], wb_ap(o), wb_ap(o))
        G_g = big_pool.tile([L, GB, L], BF, tag=f"G_{g}")
        if g % 2 == 0:
            nc.scalar.copy(G_g[:], g_ps[:])
        else:
            nc.vector.tensor_copy(G_g[:], g_ps[:])
        G_tiles.append(G_g)

    def G_ap(o):
        return G_tiles[o // GB][:, o % GB, :]

    if probe == "P2":
        _dummy_out()
        return

    # ---- u0 = sum_i x  (as (l, b) column and broadcast (o, b, l)) ----
    xbT_sb = big_pool.tile([L, B_LOC], BF)
    u0_sb = big_pool.tile([128, B_LOC, L], BF)
    for b in range(B_LOC):
        xbt_ps = psU.tile([L, 1], F32, tag="u_ps")
        for c in range(C):
            nc.tensor.matmul(xbt_ps[:], x_bf[:, b, c, :L], ones_col[:],
                             start=(c == 0), stop=(c == C - 1))
        nc.vector.tensor_copy(xbT_sb[:, b : b + 1], xbt_ps[:])
    xbrow_sb = big_pool.tile([1, B_LOC, L], BF)
    for b in range(B_LOC):
        xbrow_ps = psT.tile([1, L], BF, tag="tr")
        nc.tensor.transpose(xbrow_ps[:], xbT_sb[:, b : b + 1], ident_bf[:L, :L])
        nc.vector.tensor_copy(xbrow_sb[:, b, :], xbrow_ps[:])
    for b in range(B_LOC):
        u0_ps = psL.tile([128, L], F32, tag="lg")
        nc.tensor.matmul(u0_ps[:], ones_row[:], xbrow_sb[:, b, :])
        nc.vector.tensor_copy(u0_sb[:, b, :], u0_ps[:])

    if probe == "P3":
        _dummy_out()
        return

    # ---- routing iterations ----
    # State between iterations: uTZ (l+Z, o, b) bf16 only; u in (o,l) layout
    # is recovered at the next iteration's q-phase via 4 PE transposes that
    # run concurrently with the 128 p-step matmuls.
    uTZ_prev = None
    for t in range(1, NITER + 1):
        pT_ps = psA.tile([L, O, B_LOC], F32, tag="pT")
        for o in range(O):
            rhs = xbT_sb[:] if t == 1 else uTZ_prev[:L, o, :]
            nc.tensor.matmul(pT_ps[:, o, :], G_ap(o), rhs)
        pT_sb = sb.tile([L, O, B_LOC], BF, tag="pT_sb")
        nc.vector.tensor_copy(pT_sb[:], pT_ps[:])

        q_sb = sb.tile([128, B_LOC], F32, tag="q")
        rq_sb = sb.tile([128, B_LOC], F32, tag="rq")
        vT_sb = sb.tile([L, B_LOC, O], BF, tag="vT")
        uTZ_sb = sb.tile([L + 1, O, B_LOC], BF, tag="uT")

        # u in (o, l) layout for the q-dot
        if t == 1:
            u_all_ap = u0_sb[:]
        else:
            uall_ps = psU.tile([O, B_LOC, L], BF, tag="u_ps")
            for b in range(B_LOC):
                nc.tensor.transpose(uall_ps[:, b, :], uTZ_prev[:L, :, b],
                                    ident_bf[:L, :L])
            u_all = sb.tile([O, B_LOC, L], BF, tag="u_all")
            nc.vector.tensor_copy(u_all[:], uall_ps[:])
            u_all_ap = u_all[:]

        # q[o,b] = u . (G u)  -- p stays in PSUM; DVE/ACT read it directly
        pab_ps = psU.tile([O, B_LOC, L], BF, tag="u_ps")
        for b in range(B_LOC):
            nc.tensor.transpose(pab_ps[:, b, :], pT_sb[:, :, b],
                                ident_bf[:L, :L])
        qscr = sbE.tile([O, B_LOC, L], F32, tag="qscr")
        nc.vector.tensor_mul(qscr[:], pab_ps[:], u_all_ap)
        nc.vector.reduce_sum(q_sb[:], qscr[:], axis=mybir.AxisListType.X)

        # rq = rsqrt(q) on DVE: bit-hack + 1 Newton step (keeps ACT on a
        # single activation table: only Exp/Copy are used there)
        I32 = mybir.dt.int32
        s_i = sbE.tile([128, B_LOC], I32, tag="rs_i")
        nc.vector.tensor_scalar(out=s_i[:], in0=q_sb[:].bitcast(I32),
                                scalar1=1, scalar2=None,
                                op0=ALU.arith_shift_right)
        y0_i = sbE.tile([128, B_LOC], I32, tag="rs_y0")
        nc.vector.tensor_scalar(out=y0_i[:], in0=s_i[:], scalar1=0x5F3759DF,
                                scalar2=-1, op0=ALU.subtract, op1=ALU.mult)
        y0f = y0_i[:].bitcast(F32)
        y2 = sbE.tile([128, B_LOC], F32, tag="rs_y2")
        nc.vector.tensor_tensor(out=y2[:], in0=y0f, in1=y0f, op=ALU.mult)
        t1 = sbE.tile([128, B_LOC], F32, tag="rs_t1")
        nc.vector.tensor_tensor(out=t1[:], in0=y2[:], in1=q_sb[:], op=ALU.mult)
        t2 = sbE.tile([128, B_LOC], F32, tag="rs_t2")
        nc.vector.tensor_scalar(out=t2[:], in0=t1[:], scalar1=-0.5,
                                scalar2=1.5, op0=ALU.mult, op1=ALU.add)
        nc.vector.tensor_tensor(out=rq_sb[:], in0=y0f, in1=t2[:], op=ALU.mult)

        # v = p * rsqrt(q) for all b first (releases pab_ps before the
        # c-loops need u_ps slots)
        for b in range(B_LOC):
            v_sb = sbE.tile([O, L], BF, tag="v")
            nc.vector.tensor_scalar_mul(v_sb[:], pab_ps[:, b, :],
                                        rq_sb[:, b : b + 1])
            vt_ps = psT.tile([L, O], BF, tag="tr")
            nc.tensor.transpose(vt_ps[:], v_sb[:], ident_bf[:])
            nc.vector.tensor_copy(vT_sb[:, b, :], vt_ps[:])

        for b in range(B_LOC):
            # logits -> exp -> uT accumulation over i-chunks (uT built
            # directly; row L is Z from the ones column)
            ut2_ps = psU.tile([L + 1, O], F32, tag="u_ps")
            lg_ps = psL.tile([128, C, O], F32, tag="lg")
            for c in range(C):
                nc.tensor.matmul(lg_ps[:, c, :], xT_sb[:, b, c, :],
                                 vT_sb[:, b, :])
            exp_sb = sbE.tile([128, C, O], BF, tag="exp")
            nc.scalar.activation(exp_sb[:], lg_ps[:], AF.Exp)
            for c in range(C):
                nc.tensor.matmul(ut2_ps[:], x_bf[:, b, c, :],
                                 exp_sb[:, c, :],
                                 start=(c == 0), stop=(c == C - 1))
            nc.vector.tensor_copy(uTZ_sb[:, :, b], ut2_ps[:])

        if probe == f"I{t}":
            _dummy_out()
            return
        uTZ_prev = uTZ_sb
        if t in (1, 2):
            # WT_o = W_o^T: needed only by the final phase; emitted across
            # iterations 2 and 3 so the scheduler fills their gaps with it
            for g in range((t - 1) * O // GB // 2, t * O // GB // 2):
                wt_ps = psT.tile([L, GB, J], BF, tag="tr")
                for oo in range(GB):
                    o = g * GB + oo
                    nc.tensor.transpose(wt_ps[:, oo, :], wb_ap(o),
                                        ident_bf[:J, :J])
                if g % 2 == 0:
                    nc.scalar.copy(WT_sb[:, bass.ts(g, GB), :], wt_ps[:])
                else:
                    nc.vector.tensor_copy(WT_sb[:, bass.ts(g, GB), :], wt_ps[:])

    # rz[o,b] = 1/Z
    rz_sb = sb.tile([128, B_LOC], F32, tag="rz")
    for b in range(B_LOC):
        z_ps = psT.tile([128, 1], BF, tag="tr")
        nc.tensor.transpose(z_ps[:], uTZ_prev[L : L + 1, :, b],
                            ident_bf[L : L + 1, L : L + 1])
        zf = sbE.tile([128, 1], F32, tag="zf")
        nc.vector.tensor_copy(zf[:], z_ps[:])
        nc.vector.reciprocal(rz_sb[:, b : b + 1], zf[:])

    if probe == "I3":
        _dummy_out()
        return

    # ---- out[b,o,:] = W_o un[o,:] ----
    oT_ps = psA.tile([J, O, B_LOC], F32, tag="pT")
    for o in range(O):
        nc.tensor.matmul(oT_ps[:, o, :], WT_sb[:, o, :], uTZ_prev[:L, o, :])
    oT_sb = sb.tile([J, O, B_LOC], F32, tag="oT_sb")
    nc.scalar.copy(oT_sb[:, : O // 2, :], oT_ps[:, : O // 2, :])
    nc.vector.tensor_copy(oT_sb[:, O // 2 :, :], oT_ps[:, O // 2 :, :])
    out_all = sb.tile([O, B_LOC, J], F32, tag="out_sb")
    out_view = out_d.transpose([1, 0, 2])
    for b in range(B_LOC):
        o_ps = psT.tile([O, J], F32, tag="tr")
        nc.tensor.transpose(o_ps[:], oT_sb[:, :, b], ident_f[:J, :J])
        nc.scalar.mul(out_all[:, b, :], o_ps[:], rz_sb[:, b : b + 1])
        if b % 2 == 1:
            nc.sync.dma_start(out_view[:, b - 1 : b + 1, :],
                              out_all[:, b - 1 : b + 1, :])


def build(probe=None):
    nc = bacc.Bacc("TRN2", target_bir_lowering=False, debug=False,
                   enable_asserts=True, num_devices=N_CORES)
    x_d = nc.dram_tensor("x", [B_LOC, I, L], F32, kind="ExternalInput").ap()
    w_d = nc.dram_tensor("weight", [O, J, L], F32, kind="ExternalInput").ap()
    out_d = nc.dram_tensor("out", [B_LOC, O, J], F32, kind="ExternalOutput").ap()
    with tile.TileContext(nc) as tc:
        with ExitStack() as ctx:
            _body(ctx, tc, x_d, w_d, out_d, probe=probe)
    nc.compile()
    return nc


_NC = None
LAST_RESULTS = None  # BassKernelResults of the most recent run (for profiling)


def _get_nc():
    global _NC
    if _NC is None:
        _NC = build()
    return _NC


def kernel(x: np.ndarray, weight: np.ndarray) -> np.ndarray:
    assert x.shape == (B_GLOBAL, I, L) and weight.shape == (O, J, L)
    nc = _get_nc()
    x = np.ascontiguousarray(x, dtype=np.float32)
    weight = np.ascontiguousarray(weight, dtype=np.float32)
    in_maps = [
        {"x": x[i * B_LOC : (i + 1) * B_LOC], "weight": weight}
        for i in range(N_CORES)
    ]
    global LAST_RESULTS
    LAST_RESULTS = bass_utils.run_bass_kernel_spmd(
        nc, in_maps, core_ids=list(range(N_CORES)))
    out = np.concatenate(
        [LAST_RESULTS.results[i]["out"] for i in range(N_CORES)], axis=0)
    return out.astype(np.float32)



# revision 9
# speedup vs baseline: 1.0878x; 1.0878x over previous
"""CapsuleLinear (k-means routing) Trainium2 kernel.

Math: priors[b,o,i,j] = sum_l w[o,j,l] x[b,i,l]; 3 rounds of k-means routing
over in_capsules, squash=False.

priors is never materialized.  With G_o = W_o^T W_o (64x64 per out-capsule,
computed on-device once):

    u_0[b,l]   = sum_i x[b,i,l]                  (scale of u is irrelevant)
    per iter:  p = G_o u;  q = u.p = ||W u||^2
               rq = exp(-0.5 ln q)  (= 1/||W u||, one ACT table: Ln/Exp/Copy)
               v = p * rq           (v = W^T out_normalized)
               logits[i,o] = sum_l x[b,i,l] v[o,l]
               e = exp(logits)      (softmax Z cancels in v)
               u[o,l] = sum_i e[i,o] x[b,i,l];  Z[o] = sum_i e[i,o]
    output:    out[b,o,:] = W_o u_3[o,:] / Z_3[o]

Sharding: data-parallel over batch, 4 samples/core x 8 cores, weight
replicated, no collectives.  Host passes pre-transposed bf16 layouts
(xb with a ones column, xT, w as (j,o,l) and (l,o,j)) so the device does
no dtype conversion or weight transposition.
"""

import sys

if "/opt/trn_rl_repo" not in sys.path:
    sys.path.insert(0, "/opt/trn_rl_repo")

from contextlib import ExitStack

import ml_dtypes
import numpy as np

import concourse.bacc as bacc
import concourse.bass as bass
import concourse.bass_utils as bass_utils
import concourse.mybir as mybir
import concourse.tile as tile
from concourse.masks import make_identity

BF = mybir.dt.bfloat16
F32 = mybir.dt.float32
AF = mybir.ActivationFunctionType
ALU = mybir.AluOpType

B_GLOBAL = 32
N_CORES = 8
B = B_GLOBAL // N_CORES  # 4 samples per core
O = 128   # out_capsules
I = 512   # in_capsules
J = 64    # out_length
L = 64    # in_length
C = 4     # i-chunks of 128
NITER = 3
WCH = 8   # w DMA chunks
OCH = O // WCH


def _body(ctx: ExitStack, tc: "tile.TileContext", xb_d, xT_d, wj_d, wt_d,
          out_d, probe=None):
    nc = tc.nc

    const_pool = ctx.enter_context(tc.tile_pool(name="const", bufs=1))
    big = ctx.enter_context(tc.tile_pool(name="big", bufs=1))
    sb = ctx.enter_context(tc.tile_pool(name="sb", bufs=2))
    sbE = ctx.enter_context(tc.tile_pool(name="sbE", bufs=3))
    psP = ctx.enter_context(tc.tile_pool(name="psP", bufs=2, space="PSUM"))
    psL = ctx.enter_context(tc.tile_pool(name="psL", bufs=2, space="PSUM"))
    psU = ctx.enter_context(tc.tile_pool(name="psU", bufs=1, space="PSUM"))
    psQ = ctx.enter_context(tc.tile_pool(name="psQ", bufs=1, space="PSUM"))
    psR = ctx.enter_context(tc.tile_pool(name="psR", bufs=1, space="PSUM"))
    psB = ctx.enter_context(tc.tile_pool(name="psB", bufs=1, space="PSUM"))

    # ---- input DMAs (w chunks first: G is the long head pole) ----
    wj_tiles = []
    for k in range(WCH):
        wj_k = big.tile([J, OCH, L], BF, tag=f"wj_{k}")
        nc.sync.dma_start(wj_k[:], wj_d[:, bass.ts(k, OCH), :])
        wj_tiles.append(wj_k)
    xb_sb = big.tile([128, B, C, L + 1], BF)
    nc.sync.dma_start(xb_sb[:], xb_d)
    xT_sb = big.tile([L, B, C, 128], BF)
    nc.sync.dma_start(xT_sb[:], xT_d)
    wt_sb = big.tile([L, O, J], BF)
    nc.sync.dma_start(wt_sb[:], wt_d)

    def wj_ap(o):
        return wj_tiles[o // OCH][:, o % OCH, :]

    # ---- constants ----
    ident_bf = const_pool.tile([128, 128], BF)
    make_identity(nc, ident_bf[:])
    ident_f = const_pool.tile([128, 128], F32)
    make_identity(nc, ident_f[:])
    ones_col128 = const_pool.tile([128, 1], BF)
    nc.vector.memset(ones_col128[:], 1.0)
    ones_row = const_pool.tile([1, L], BF)
    nc.vector.memset(ones_row[:], 1.0)
    ones_col64 = ones_col128[:L, :]

    # ---- G_o = W_o^T W_o, chunk-pipelined behind the w DMA ----
    GB = 8
    G_tiles = []
    for k in range(O // GB):
        g_ps = psL.tile([L, GB, L], F32, tag="lg")
        for i in range(GB):
            nc.tensor.matmul(g_ps[:, i, :], wj_ap(k * GB + i),
                             wj_ap(k * GB + i))
        G_k = big.tile([L, GB, L], BF, tag=f"G_{k}")
        if k % 2 == 0:
            nc.vector.tensor_copy(G_k[:], g_ps[:])
        else:
            nc.scalar.copy(G_k[:], g_ps[:])
        G_tiles.append(G_k)

    def G_ap(o):
        return G_tiles[o // GB][:, o % GB, :]

    # ---- u0[l, b] = sum_i x ----
    u0_ps = psQ.tile([L, B], F32, tag="q")
    for b in range(B):
        for c in range(C):
            nc.tensor.matmul(u0_ps[:, b : b + 1], xb_sb[:, b, c, :L],
                             ones_col128[:], start=(c == 0), stop=(c == C - 1))
    u0_sb = sbE.tile([L, B], BF, tag="u0")
    nc.vector.tensor_copy(u0_sb[:], u0_ps[:])

    def _dummy_out():
        nc.sync.dma_start(out_d[0], ident_f[:1, :J])

    if probe == "P2":
        _dummy_out()
        return

    # ---- routing iterations; uTZ (l+Z, o, b) bf16 is the carried state ----
    uTZ_prev = None
    for t in range(1, NITER + 1):
        # p = G u  -> (l, o, b)
        pT_ps = psP.tile([L, O, B], F32, tag="pT")
        for o in range(O):
            rhs = u0_sb[:] if t == 1 else uTZ_prev[:L, o, :]
            nc.tensor.matmul(pT_ps[:, o, :], G_ap(o), rhs)
        pT_sb = sb.tile([L, O, B], BF, tag="pT_sb")
        nc.vector.tensor_copy(pT_sb[:], pT_ps[:])

        # q[o, b] = sum_l p*u via per-b column matmuls
        q_ps = psQ.tile([O, B], F32, tag="q")
        if t == 1:
            for b in range(B):
                nc.tensor.matmul(q_ps[:, b : b + 1], pT_sb[:, :, b],
                                 u0_sb[:, b : b + 1])
        else:
            qscr = sbE.tile([L, O, B], BF, tag="qscr")
            nc.vector.tensor_tensor(out=qscr[:], in0=pT_ps[:],
                                    in1=uTZ_prev[:L], op=ALU.mult)
            for b in range(B):
                nc.tensor.matmul(q_ps[:, b : b + 1], qscr[:, :, b],
                                 ones_col64)

        # rq = exp(-0.5 ln q) = 1/||W u||   (single ACT table)
        lnq = sbE.tile([O, B], F32, tag="lnq")
        nc.scalar.activation(lnq[:], q_ps[:], AF.Ln)
        rq = sbE.tile([O, B], BF, tag="rq")
        nc.scalar.activation(rq[:], lnq[:], AF.Exp, scale=-0.5)

        # broadcast rq over l: per-b transpose to a row, then ones-col matmul
        rqT_ps = psR.tile([1, B, O], BF, tag="rqT")
        for b in range(B):
            nc.tensor.transpose(rqT_ps[:, b, :], rq[:, b : b + 1],
                                ident_bf[:])
        rqT_sb = sbE.tile([1, B, O], BF, tag="rqTs")
        nc.vector.tensor_copy(rqT_sb[:], rqT_ps[:])
        rqb_ps = psB.tile([L, B, O], F32, tag="rqb")
        for b in range(B):
            nc.tensor.matmul(rqb_ps[:, b, :], ones_row[:],
                             rqT_sb[:, b, :])

        # v = p * rq  -> (l, o, b)
        v_sb = sb.tile([L, O, B], BF, tag="v")
        nc.vector.tensor_tensor(out=v_sb[:], in0=pT_sb[:],
                                in1=rqb_ps[:].transpose([0, 2, 1]),
                                op=ALU.mult)

        # logits -> exp -> u accumulation (Z rides along as row L)
        lg_tiles = []
        for b in range(B):
            lg_ps = psL.tile([128, C, O], F32, tag="lg")
            for c in range(C):
                nc.tensor.matmul(lg_ps[:, c, :], xT_sb[:, b, c, :],
                                 v_sb[:, :, b])
            lg_tiles.append(lg_ps)
        exp_tiles = []
        for b in range(B):
            exp_sb = sbE.tile([128, C, O], BF, tag=f"exp{b % 2}")
            nc.scalar.activation(exp_sb[:], lg_tiles[b][:], AF.Exp)
            exp_tiles.append(exp_sb)
        u_ps = psU.tile([L + 1, B, O], F32, tag="u")
        for b in range(B):
            for c in range(C):
                nc.tensor.matmul(u_ps[:, b, :], xb_sb[:, b, c, :],
                                 exp_tiles[b][:, c, :],
                                 start=(c == 0), stop=(c == C - 1))
        uTZ_sb = sb.tile([L + 1, O, B], BF, tag="uT")
        for b in range(B):
            nc.vector.tensor_copy(uTZ_sb[:, :, b], u_ps[:, b, :])

        if probe == f"I{t}":
            _dummy_out()
            return
        uTZ_prev = uTZ_sb

    # ---- rz = 1/Z ----
    z_ps = psR.tile([O, B, 2], BF, tag="rqT")
    for b in range(B):
        nc.tensor.transpose(z_ps[:, b, 0:1], uTZ_prev[L : L + 1, :, b],
                            ident_bf[L : L + 1, L : L + 1])
    rz_sb = sbE.tile([O, B], F32, tag="rz")
    nc.vector.reciprocal(rz_sb[:], z_ps[:, :, 0])

    # ---- out[b,o,:] = W_o u_3[o,:] / Z ----
    oT_ps = psP.tile([J, O, B], F32, tag="pT")
    for o in range(O):
        nc.tensor.matmul(oT_ps[:, o, :], wt_sb[:, o, :], uTZ_prev[:L, o, :])
    oT_sb = sb.tile([J, O, B], F32, tag="oT_sb")
    nc.scalar.copy(oT_sb[:, : O // 2, :], oT_ps[:, : O // 2, :])
    nc.vector.tensor_copy(oT_sb[:, O // 2 :, :], oT_ps[:, O // 2 :, :])
    out_all = sb.tile([O, B, J], F32, tag="out_sb")
    out_view = out_d.transpose([1, 0, 2])
    for b in range(B):
        o_ps = psB.tile([O, J], F32, tag="rqb")
        nc.tensor.transpose(o_ps[:], oT_sb[:, :, b], ident_f[:J, :J])
        nc.scalar.mul(out_all[:, b, :], o_ps[:], rz_sb[:, b : b + 1])
        if b % 2 == 1:
            nc.sync.dma_start(out_view[:, b - 1 : b + 1, :],
                              out_all[:, b - 1 : b + 1, :])


def build(probe=None):
    nc = bacc.Bacc("TRN2", target_bir_lowering=False, debug=False,
                   enable_asserts=True, num_devices=N_CORES)
    xb_d = nc.dram_tensor("xb", [128, B, C, L + 1], BF, kind="ExternalInput").ap()
    xT_d = nc.dram_tensor("xT", [L, B, C, 128], BF, kind="ExternalInput").ap()
    wj_d = nc.dram_tensor("wj", [J, O, L], BF, kind="ExternalInput").ap()
    wt_d = nc.dram_tensor("wt", [L, O, J], BF, kind="ExternalInput").ap()
    out_d = nc.dram_tensor("out", [B, O, J], F32, kind="ExternalOutput").ap()
    with tile.TileContext(nc) as tc:
        with ExitStack() as ctx:
            _body(ctx, tc, xb_d, xT_d, wj_d, wt_d, out_d, probe=probe)
    nc.compile()
    return nc


_NC = None
LAST_RESULTS = None


def _get_nc():
    global _NC
    if _NC is None:
        _NC = build()
    return _NC


def kernel(x: np.ndarray, weight: np.ndarray) -> np.ndarray:
    assert x.shape == (B_GLOBAL, I, L) and weight.shape == (O, J, L)
    nc = _get_nc()
    bf16 = ml_dtypes.bfloat16
    x = np.ascontiguousarray(x, dtype=np.float32)
    w = np.ascontiguousarray(weight, dtype=np.float32)
    wj = np.ascontiguousarray(w.transpose(1, 0, 2).astype(bf16))   # (j, o, l)
    wt = np.ascontiguousarray(w.transpose(2, 0, 1).astype(bf16))   # (l, o, j)
    in_maps = []
    for i in range(N_CORES):
        xs = x[i * B : (i + 1) * B]                  # (B, I, L)
        xr = xs.reshape(B, 128, C, L)                # i = 4p + c
        xb = np.empty((128, B, C, L + 1), dtype=bf16)
        xb[..., :L] = xr.transpose(1, 0, 2, 3).astype(bf16)
        xb[..., L] = 1.0
        xT = np.ascontiguousarray(xr.transpose(3, 0, 2, 1).astype(bf16))
        in_maps.append({"xb": xb, "xT": xT, "wj": wj, "wt": wt})
    global LAST_RESULTS
    LAST_RESULTS = bass_utils.run_bass_kernel_spmd(
        nc, in_maps, core_ids=list(range(N_CORES)))
    out = np.concatenate(
        [LAST_RESULTS.results[i]["out"] for i in range(N_CORES)], axis=0)
    return out.astype(np.float32)
